# revision 91
# baseline (speedup 1.0000x reference)
"""TRN2 Bass kernel for CustomMultiHeadAttention (B=2, S=2048, D=1024, H=16).

Sharding: batch x head-group over 8 cores. Core c handles batch c//4 and
heads [4*(c%4), 4*(c%4)+4). Wq/Wk/Wv are column-sliced per head group
(Megatron column-parallel), Wo row-sliced (row-parallel); the partial
outputs are summed with an on-device ReduceScatter over each batch's
4-core group, so every core returns a distinct [512, 1024] slice of the
final output. Host concatenates and adds bo.

Device dataflow (per core):
  x arrives in bf16 and is transposed straight out of DRAM into the
  [e, s] SBUF layout by xbar DMA-transposes with 3D output access
  patterns (one DMA per s-row block; no PE/PSUM/drain involvement).
  Wk/Wq are host-packed into one SBUF-layout tensor so a single DMA
  grant covers both (the cost model serializes DMA grants at ~2.2us
  each, so the prologue uses few, large, need-ordered transfers).
  Q_T/K_T = W^T x^T accumulate in PSUM and drain to bf16 via DVE
  (plain copies when biases are zero, so nothing gates on their load);
  V = x Wv lands in an augmented layout with a ones column so P@V_aug
  row sums appear next to the outputs (early V drains run on ScalarE,
  which is idle before the first exp). Scores per (head, 256-row macro
  block) in S^T layout (k on partitions); the fully-masked first
  q-half of the diagonal k-tile is skipped. exp on ScalarE; causal
  zeroing of the two triangular half-blocks via GPSIMD multiplies with
  a [128,128] 0/1 tile. PV is computed in [q, d] layout (stationary =
  P^T chunk, moving = V): 65-wide matmuls accumulating over k make the
  softmax divisor a per-partition scalar (DVE reciprocal + multiply,
  writing A in bf16). A is PE-transposed back (bf16 identity) for the
  row-parallel output projection; o_sb drains on DVE (ScalarE for the
  last block's tail). The mb loop is software-pipelined: the next
  macro block's first score/exp pass is emitted inside the current
  block's PV tail so ScalarE never idles at block boundaries, and
  K/Q/V projection fill-units pad the PE stream under a deadline/cap
  schedule. Partial outputs are stored in bf16 (gpsimd SWDGE mid-loop,
  scalar HWDGE for the split final store) and ReduceScattered; the
  final f32 output is produced by a casting gpsimd DMA (issued early
  in the non-collective timed build, once its source rows are stored).
"""

import ml_dtypes
import numpy as np

import concourse.bass as bass
import concourse.mybir as mybir
import concourse.tile as tile
from concourse import bacc
from concourse.bass_utils import run_bass_kernel_spmd

B, S, D, H, HD = 2, 2048, 1024, 16, 64
NCORES = 8
HPC = 4                  # heads per core
DH = HPC * HD            # 256: per-core slice of the model dim
SOUT = S // 4            # 512: rows per core after reduce-scatter
NEG = -1.0e9
NST = S // 128           # 16 sequence tiles
NET = D // 128           # 8 embedding tiles
NMB = S // 256           # 8 macro q-blocks
VW = HPC * 65            # per-st width of the augmented-V layout

F32 = mybir.dt.float32
F32R = mybir.dt.float32r
BF16 = mybir.dt.bfloat16
AF = mybir.ActivationFunctionType
ALU = mybir.AluOpType

_BUILD_CACHE = {}


def _build(apply_mask: bool, apply_bv: bool, apply_bqk: bool = False,
           collective: bool = True):
    key = (apply_mask, apply_bv, apply_bqk, collective)
    if key in _BUILD_CACHE:
        return _BUILD_CACHE[key]

    nc = bacc.Bacc("TRN2", target_bir_lowering=False, debug=False,
                   num_devices=NCORES if collective else 1)

    x_d = nc.dram_tensor("x", [S, D], BF16, kind="ExternalInput").ap()
    # wk and wq pre-arranged by the host into the exact SBUF layout and
    # packed into one tensor: a single DMA grant covers both
    wkq_d = nc.dram_tensor("wkq", [128, 2 * NET * DH], BF16,
                           kind="ExternalInput").ap()
    wv_d = nc.dram_tensor("wv", [D, DH], BF16, kind="ExternalInput").ap()
    wo_d = nc.dram_tensor("wo", [DH, D], BF16, kind="ExternalInput").ap()
    bq_d = bk_d = None
    if apply_bqk:
        bq_d = nc.dram_tensor("bq", [DH], F32, kind="ExternalInput").ap()
        bk_d = nc.dram_tensor("bk", [DH], F32, kind="ExternalInput").ap()
    identb_d = nc.dram_tensor("identb", [128, 128], BF16,
                              kind="ExternalInput").ap()
    tri_d = nc.dram_tensor("tri01", [128, 128], BF16, kind="ExternalInput").ap()
    bv_d = madd_d = None
    if apply_bv:
        bv_d = nc.dram_tensor("bv", [DH], BF16, kind="ExternalInput").ap()
    if apply_mask:
        madd_d = nc.dram_tensor("madd", [S], F32, kind="ExternalInput").ap()
    out_d = nc.dram_tensor("out", [SOUT, D], F32, kind="ExternalOutput").ap()

    with tile.TileContext(nc) as tc:
        with (
            tc.tile_pool(name="persist", bufs=1) as pp,
            tc.tile_pool(name="dram", bufs=1, space="DRAM") as dp,
        ):
            # persistent SBUF tensors
            qT = [pp.tile([128, S], BF16, name=f"qT{i}") for i in range(2)]
            kT = [pp.tile([128, S], BF16, name=f"kT{i}") for i in range(2)]
            # both jt slices in one tile so a single xbar DMA transpose can
            # scatter across them with a 3D access pattern
            aT_all = pp.tile([128, 2 * S], BF16, name="aT_all")
            identb_t = pp.tile([128, 128], BF16, name="identb_t")
            vug = pp.tile([128, NST * VW], BF16, name="vug")
            ones_b = pp.tile([1, 128], BF16, name="ones_b")
            tri_t = pp.tile([128, 128], BF16, name="tri_t")
            bq_t = bk_t = None
            if apply_bqk:
                bq_t = pp.tile([128, 2], F32, name="bq_t")
                bk_t = pp.tile([128, 2], F32, name="bk_t")

            o_part = dp.tile([S, D], BF16, name="o_part")
            rs_out = dp.tile([SOUT, D], BF16, name="rs_out")
            with (
                tc.tile_pool(name="stageb", bufs=1) as sp,
                tc.tile_pool(name="attn_sb", bufs=2) as ap,
                tc.tile_pool(name="o_sbp", bufs=8) as op_,
                tc.tile_pool(name="psu", bufs=2, space="PSUM") as psu,
            ):
                xT = sp.tile([128, NET * S], BF16, name="xT")  # [e, et*S + s]
                wkq_t = sp.tile([128, 2 * NET * DH], BF16, name="wkq_t")
                wk_t = wkq_t[:, 0:NET * DH]
                wq_t = wkq_t[:, NET * DH:2 * NET * DH]
                wv_t = sp.tile([128, NET * DH], BF16, name="wv_t")
                wo_t = sp.tile([128, 2 * D], BF16, name="wo_t")

                # one xbar DMA transpose moves an s-row-block of x straight
                # from DRAM into the [e, s] SBUF layout: the 3D output AP
                # scatters the 8 transposed e-tiles across xT (SP queue)
                def x_tp(s0, s1, eng=None):
                    (eng or nc.sync).dma_start(
                        xT[:].rearrange("p (et s) -> p et s", s=S)
                          [:, :, s0:s1],
                        x_d[s0:s1, :],
                        transpose=True,
                    )

                # DMA grants serialize in the cost model (~2.2us latency
                # per grant), so the prologue is ordered by first need and
                # uses as few, as-large transfers as possible
                nc.sync.dma_start(tri_t[:], tri_d[:])
                x_tp(0, 256)
                x_tp(256, 512)
                nc.scalar.dma_start(wkq_t[:], wkq_d[:])
                nc.sync.dma_start(
                    wv_t[:].rearrange("p (et j) -> p et j", j=DH),
                    wv_d.rearrange("(et p) j -> p et j", p=128),
                )
                if apply_bqk:
                    nc.scalar.dma_start(
                        bq_t[:], bq_d.rearrange("(t p) -> p t", p=128))
                    nc.scalar.dma_start(
                        bk_t[:], bk_d.rearrange("(t p) -> p t", p=128))
                nc.sync.dma_start(
                    wo_t[:].rearrange("p (t m) -> p t m", m=D),
                    wo_d.rearrange("(t p) m -> p t m", p=128),
                )
                nc.sync.dma_start(identb_t[:], identb_d[:])
                nc.vector.memset(ones_b[:], 1.0)
                # ones columns of the augmented-V layout
                nc.vector.memset(
                    vug[:].rearrange("p (st h c) -> p st h c", h=HPC, c=65)
                       [:, :, :, 64:65], 1.0,
                )
                bv_t = None
                if apply_bv:
                    bv_t = pp.tile([1, DH], BF16, name="bv_t")
                    nc.sync.dma_start(bv_t[:], bv_d[None, :])
                madd_t = None
                if apply_mask:
                    madd_t = pp.tile([128, NST], F32, name="madd_t")
                    nc.sync.dma_start(
                        madd_t[:], madd_d.rearrange("(t p) -> p t", p=128)
                    )

                # projections, interleaved so early attention unblocks fast
                def kq_chunk(tgt, w_t, b_t, jt, s0, s1, nm):
                    q_ps = psu.tile([128, 512], F32,
                                    name=f"{nm}_ps{jt}_{s0}", tag="small",
                                    bufs=2)
                    for et in range(NET):
                        nc.tensor.matmul(
                            q_ps[:, 0:s1 - s0],
                            w_t[:, et * DH + jt * 128:
                                et * DH + (jt + 1) * 128],
                            xT[:, et * S + s0:et * S + s1],
                            start=(et == 0), stop=(et == NET - 1),
                        )
                    # bias-add + PSUM->SBUF drain on DVE (plain copy when
                    # the biases are zero, so nothing gates on their load)
                    if apply_bqk:
                        nc.vector.tensor_scalar(
                            tgt[jt][:, s0:s1],
                            q_ps[:, 0:s1 - s0], b_t[:, jt:jt + 1],
                            None, ALU.add,
                        )
                    else:
                        nc.vector.tensor_copy(
                            tgt[jt][:, s0:s1], q_ps[:, 0:s1 - s0],
                        )

                def v_tile(st):
                    v_ps = psu.tile([128, DH], F32, name=f"v_ps{st}",
                                    tag="small", bufs=2)
                    last = NET - 1 if not apply_bv else -1
                    for et in range(NET):
                        nc.tensor.matmul(
                            v_ps[:],
                            xT[:, et * S + st * 128:et * S + (st + 1) * 128],
                            wv_t[:, et * DH:(et + 1) * DH],
                            start=(et == 0), stop=(et == last),
                        )
                    if apply_bv:
                        nc.tensor.matmul(
                            v_ps[:], ones_b[:], bv_t[:],
                            start=False, stop=True,
                        )
                    veng = nc.scalar.copy if st < 8 else nc.vector.tensor_copy
                    veng(
                        vug[:, st * VW:(st + 1) * VW]
                           .rearrange("p (h c) -> p h c", c=65)[:, :, 0:64],
                        v_ps[:].rearrange("p (h c) -> p h c", c=64),
                    )

                # prologue: the projections mb=0 needs, narrow first
                # chunks so PE starts as soon as x rows 0:256 land
                for jt in range(2):
                    kq_chunk(kT, wk_t, bk_t, jt, 0, 256, "k")
                for jt in range(2):
                    kq_chunk(qT, wq_t, bq_t, jt, 0, 256, "q")
                v_tile(0)

                # remaining fill units, drained between attention emissions
                # so the PE stream always has independent work during exp.
                def kq(jt, cn):
                    kq_chunk(kT, wk_t, bk_t, jt, cn * 512, (cn + 1) * 512, "k")
                    kq_chunk(qT, wq_t, bq_t, jt, cn * 512, (cn + 1) * 512, "q")

                def kq_rest():
                    # second half of the narrow prologue chunks
                    for jt in range(2):
                        kq_chunk(kT, wk_t, bk_t, jt, 256, 512, "k")
                    for jt in range(2):
                        kq_chunk(qT, wq_t, bq_t, jt, 256, 512, "q")

                units = []
                units += [lambda: v_tile(1)]                           # -
                units += [lambda: kq_rest()]                           # 0
                units += [lambda: x_tp(512, 1024),
                          lambda: v_tile(2), lambda: v_tile(3)]        # 1-3
                units += [lambda: kq(0, 1), lambda: kq(1, 1)]          # 4-5
                units += [lambda: v_tile(4), lambda: v_tile(5)]        # 6-7
                units += [lambda: x_tp(1024, 1536),
                          lambda: v_tile(6), lambda: v_tile(7)]        # 8-10
                units += [lambda: kq(0, 2), lambda: kq(1, 2)]          # 11-12
                units += [lambda: v_tile(8), lambda: v_tile(9)]        # 13-14
                units += [lambda: x_tp(1536, 2048),
                          lambda: v_tile(10), lambda: v_tile(11)]      # 15-17
                units += [lambda: kq(0, 3), lambda: kq(1, 3)]          # 18-19
                units += [lambda: v_tile(12), lambda: v_tile(13)]      # 20-21
                units += [lambda: v_tile(14), lambda: v_tile(15)]      # 22-23
                # hard deadlines: units that must be emitted before mb starts
                # (scores mb needs kq cn <= mb//2 and the prologue rest, pv
                # mb needs v < 2mb+2 with the last one allowed to slip to
                # the first in-mb drain slot, which runs before pv)
                DL = {1: 4, 2: 8, 3: 11, 4: 15, 5: 18, 6: 22, 7: 24}
                # units needed before the pipelined s_exp_pair(0, mb+1)
                DLS = {1: 2, 2: 7, 3: 7, 4: 14, 5: 14, 6: 21, 7: 21}
                # soft targets: cap eager in-mb draining so some PE fill
                # work remains for the ACT-bound late macro blocks
                CUM = {0: 5, 1: 9, 2: 11, 3: 14, 4: 16, 5: 19, 6: 21, 7: 25}
                ndrained = [0]

                def drain(n):
                    take = min(n, len(units) - ndrained[0])
                    for _ in range(take):
                        units[ndrained[0]]()
                        ndrained[0] += 1

                def drain_to(idx):
                    drain(max(0, idx - ndrained[0]))

                pt_tiles = {}
                asb_tiles = {}

                def s_exp_pair(jt, mb):
                    nks = 2 * (mb + 1)
                    if True:
                        pts = []
                        for h in (2 * jt, 2 * jt + 1):
                            pt_t = ap.tile([128, 4096], BF16,
                                           name=f"pt{h}_{mb}", tag="pt",
                                           bufs=6)
                            pt_tiles[(mb, h)] = pt_t
                            pts.append(pt_t)
                        for g in range((nks + 3) // 4):
                            ks, ke = g * 4, min(nks, g * 4 + 4)
                            sts = [
                                psu.tile([128, 1024], F32,
                                         name=f"st{h}_{mb}_{g}",
                                         tag="big", bufs=2)
                                for h in (2 * jt, 2 * jt + 1)
                            ]
                            for kt in range(ks, ke):
                                c0 = (kt - ks) * 256
                                for i, hp in enumerate((0, 64)):
                                    nc.tensor.matmul(
                                        sts[i][:, c0:c0 + 256],
                                        kT[jt][hp:hp + 64,
                                               kt * 128:(kt + 1) * 128],
                                        qT[jt][hp:hp + 64,
                                               mb * 256:(mb + 1) * 256],
                                        start=True, stop=True,
                                    )
                            for i in range(2):
                                if apply_mask:
                                    for kt in range(ks, ke):
                                        c0 = (kt - ks) * 256
                                        if kt == nks - 1:
                                            # upper half-block fully masked
                                            nc.scalar.activation(
                                                pts[i][:, kt * 256 + 128:
                                                       (kt + 1) * 256],
                                                sts[i][:, c0 + 128:c0 + 256],
                                                AF.Exp,
                                                bias=madd_t[:, kt:kt + 1],
                                            )
                                        else:
                                            nc.scalar.activation(
                                                pts[i][:, kt * 256:
                                                       (kt + 1) * 256],
                                                sts[i][:, c0:c0 + 256],
                                                AF.Exp,
                                                bias=madd_t[:, kt:kt + 1],
                                            )
                                else:
                                    nc.scalar.activation(
                                        pts[i][:, ks * 256:ke * 256],
                                        sts[i][:, 0:(ke - ks) * 256], AF.Exp,
                                    )
                        # causal mask: zero the two triangular half-blocks
                        # (exp(S+C) == exp(S) * M, M in {0,1}); the fully
                        # masked half-block of the last ksub is skipped by
                        # both the exp above and PV
                        for i in range(2):
                            nc.gpsimd.tensor_mul(
                                pts[i][:, (nks - 2) * 256:
                                       (nks - 2) * 256 + 128],
                                pts[i][:, (nks - 2) * 256:
                                       (nks - 2) * 256 + 128],
                                tri_t[:],
                            )
                            nc.gpsimd.tensor_mul(
                                pts[i][:, (nks - 1) * 256 + 128:
                                       nks * 256],
                                pts[i][:, (nks - 1) * 256 + 128:
                                       nks * 256],
                                tri_t[:],
                            )

                def pv_div(h, mb):
                    nks = 2 * (mb + 1)
                    if True:
                        # PV in [q, d] layout: stationary = P^T chunk
                        # (k on partitions), moving = augmented V (65 cols).
                        # Row sums land in column 64, so the softmax divide
                        # is a per-partition tensor_scalar on DVE. Both
                        # q-subtiles share one PSUM bank (columns qs*65).
                        pt_t = pt_tiles.pop((mb, h))
                        a_ps = psu.tile([128, 130], F32, name=f"a{h}_{mb}",
                                        tag="aps", bufs=2)
                        for qs in range(2):
                            if (mb, qs) not in asb_tiles:
                                asb_tiles[(mb, qs)] = ap.tile(
                                    [128, DH], BF16, name=f"asb{qs}_{mb}",
                                    tag="asb", bufs=4)
                            a_sb = asb_tiles[(mb, qs)]
                            last = nks - 1 if qs == 1 else nks - 2
                            for kt in range(last + 1):
                                nc.tensor.matmul(
                                    a_ps[:, qs * 65:(qs + 1) * 65],
                                    pt_t[:, kt * 256 + qs * 128:
                                         kt * 256 + qs * 128 + 128],
                                    vug[:, kt * VW + h * 65:
                                        kt * VW + (h + 1) * 65],
                                    start=(kt == 0), stop=(kt == last),
                                )
                            zr = ap.tile([128, 1], F32, name=f"zr{h}_{qs}_{mb}",
                                         tag="zr", bufs=4)
                            with nc.allow_low_precision(reason="softmax recip"):
                                nc.vector.reciprocal(
                                    zr[:], a_ps[:, qs * 65 + 64:qs * 65 + 65])
                            nc.vector.tensor_scalar(
                                a_sb[:, h * 64:(h + 1) * 64],
                                a_ps[:, qs * 65:qs * 65 + 64],
                                zr[:], None, ALU.mult,
                            )

                def at_pair(jt, mb):
                    if True:
                        # transpose A back to [d, s] for the output proj on
                        # the PE (bf16 input: 1 cycle/row); both q-subtiles
                        # land in one PSUM bank, drained by a single DVE copy
                        at_ps = psu.tile([128, 256], BF16,
                                         name=f"atp{jt}_{mb}",
                                         tag="aps", bufs=2)
                        for qs in range(2):
                            nc.tensor.transpose(
                                at_ps[:, qs * 128:(qs + 1) * 128],
                                asb_tiles[(mb, qs)][:, jt * 128:(jt + 1) * 128],
                                identb_t[:],
                            )
                        nc.vector.tensor_copy(
                            aT_all[:, jt * S + mb * 256:
                                   jt * S + (mb + 1) * 256], at_ps[:],
                        )

                o_sbs = {}

                def wo_st(pmb, i):
                    if True:
                        # one s-tile of the previous mb's output projection
                        if i == 0:
                            o_sbs[pmb] = op_.tile([128, 2 * D], BF16,
                                                  name=f"o_sb{pmb}", tag="osb")
                        o_sb = o_sbs[pmb]
                        st = 2 * pmb + i
                        for mc in range(2):
                            o_ps = psu.tile([128, 512], F32,
                                            name=f"o_ps{st}_{mc}",
                                            tag="small", bufs=2)
                            for jjt in range(2):
                                nc.tensor.matmul(
                                    o_ps[:],
                                    aT_all[:, jjt * S + st * 128:
                                           jjt * S + (st + 1) * 128],
                                    wo_t[:, jjt * D + mc * 512:
                                         jjt * D + (mc + 1) * 512],
                                    start=(jjt == 0), stop=(jjt == 1),
                                )
                            # o drains on DVE (GPSIMD cannot read PSUM);
                            # the last pmb alternates with ScalarE, idle
                            # once the exps are done, to shorten the tail
                            if pmb == NMB - 1 and mc == 1:
                                nc.scalar.copy(
                                    o_sb[:, i * D + mc * 512:
                                         i * D + (mc + 1) * 512], o_ps[:]
                                )
                            else:
                                nc.vector.tensor_copy(
                                    o_sb[:, i * D + mc * 512:
                                         i * D + (mc + 1) * 512], o_ps[:]
                                )
                        if pmb == NMB - 1:
                            # split store so the final DMA is half-size; on
                            # the gpsimd queue so the ReduceScatter (same
                            # queue, in-order) cannot race the stores
                            nc.gpsimd.dma_start(
                                o_part[(2 * pmb + i) * 128:
                                       (2 * pmb + i + 1) * 128, :],
                                o_sb[:, i * D:(i + 1) * D],
                            )
                        elif i == 1:
                            nc.gpsimd.dma_start(
                                o_part[2 * pmb * 128:(2 * pmb + 2) * 128, :]
                                .rearrange("(t p) m -> p t m", p=128),
                                o_sb[:].rearrange("p (t m) -> p t m", m=D),
                            )
                            if not collective and pmb == 1:
                                # the timed build's output slice is rows
                                # 0:512, fully written once pmb 0-1 stored
                                nc.gpsimd.dma_start(
                                    out_d[:], o_part[0:SOUT, :])

                def wo_mb(pmb):
                    wo_st(pmb, 0)
                    wo_st(pmb, 1)

                # software pipeline: the next mb's first score/exp pass is
                # emitted inside this mb's PV tail so ScalarE never idles at
                # mb boundaries; within an mb the next head pair's scores
                # precede the previous pair's PV; Wo work (deferred up to
                # several mbs, it only needs the persistent aT) and
                # projection fill-units pad the PE stream further, weighted
                # toward the exp-bound late macro blocks.
                WOP = {2: [0], 6: [1, 2], 7: [3, 4, 5, 6]}
                for mb in range(NMB):
                    cap = CUM.get(mb, len(units))

                    def drain1(cap=cap):
                        if ndrained[0] < cap:
                            drain(1)

                    woq = [(p, i) for p in WOP.get(mb, []) for i in (0, 1)]

                    def wo_next(n=1):
                        for _ in range(min(n, len(woq))):
                            p, i = woq.pop(0)
                            wo_st(p, i)

                    drain_to(DL.get(mb, 0))
                    if mb == 0:
                        s_exp_pair(0, 0)
                    wo_next()
                    drain1()
                    s_exp_pair(1, mb)
                    pv_div(0, mb)
                    wo_next()
                    drain1()
                    if mb + 1 < NMB:
                        drain_to(DLS.get(mb + 1, 0))
                        s_exp_pair(0, mb + 1)
                    pv_div(1, mb)
                    wo_next()
                    drain1()
                    pv_div(2, mb)
                    at_pair(0, mb)
                    wo_next()
                    drain1()
                    pv_div(3, mb)
                    wo_next()
                    at_pair(1, mb)
                    drain1()
                    wo_next(len(woq))
                wo_mb(NMB - 1)
                if collective:
                    nc.gpsimd.collective_compute(
                        "ReduceScatter", ALU.add,
                        replica_groups=[[0, 1, 2, 3], [4, 5, 6, 7]],
                        ins=[o_part.opt()],
                        outs=[rs_out.opt()],
                    )
                    nc.gpsimd.dma_start(out_d[:], rs_out[:])
                # (non-collective final out DMA already issued after pmb 1)

    nc.compile()
    _BUILD_CACHE[key] = nc
    return nc


def _run(inputs, trace=False, trace_cores=None):
    x = np.asarray(inputs["x"], np.float32)
    mask = np.asarray(inputs["mask"], np.float32)
    Wq = np.asarray(inputs["Wq"], np.float32)
    bq = np.asarray(inputs["bq"], np.float32)
    Wk = np.asarray(inputs["Wk"], np.float32)
    bk = np.asarray(inputs["bk"], np.float32)
    Wv = np.asarray(inputs["Wv"], np.float32)
    bv = np.asarray(inputs["bv"], np.float32)
    Wo = np.asarray(inputs["Wo"], np.float32)
    bo = np.asarray(inputs["bo"], np.float32)

    apply_mask = not np.all(mask == 1.0)
    apply_bv = bool(np.any(bv))
    apply_bqk = bool(np.any(bq)) or bool(np.any(bk))
    nc = _build(apply_mask, apply_bv, apply_bqk)

    scale = np.float32(1.0 / np.sqrt(HD))
    # triangular keep-mask for a diagonal 128x128 half-block: keep k <= q
    tri01 = (np.arange(128)[:, None] <= np.arange(128)[None, :]).astype(
        np.float32)

    in_maps = []
    for c in range(NCORES):
        b, hg = c // 4, c % 4
        js = slice(hg * DH, (hg + 1) * DH)
        def sb_layout(w):
            # [D, DH] -> [128, NET*DH] rows matching the on-chip w tiles
            return w.reshape(NET, 128, DH).transpose(1, 0, 2).reshape(
                128, NET * DH)

        wkq = np.concatenate(
            [sb_layout(Wk[:, js]), sb_layout(Wq[:, js] * scale)], axis=1)
        m = {
            "x": np.ascontiguousarray(x[b].astype(ml_dtypes.bfloat16)),
            "wkq": np.ascontiguousarray(wkq.astype(ml_dtypes.bfloat16)),
            "wv": np.ascontiguousarray(Wv[:, js].astype(ml_dtypes.bfloat16)),
            "wo": np.ascontiguousarray(Wo[js, :].astype(ml_dtypes.bfloat16)),
            "identb": np.eye(128, dtype=np.float32).astype(ml_dtypes.bfloat16),
            "tri01": tri01.astype(ml_dtypes.bfloat16),
        }
        if apply_bqk:
            m["bq"] = np.ascontiguousarray(bq[js]) * scale
            m["bk"] = np.ascontiguousarray(bk[js])
        if apply_bv:
            m["bv"] = np.ascontiguousarray(bv[js].astype(ml_dtypes.bfloat16))
        if apply_mask:
            m["madd"] = np.ascontiguousarray((1.0 - mask[b]) * NEG)
        in_maps.append(m)

    res = run_bass_kernel_spmd(
        nc, in_maps, list(range(NCORES)), trace=trace, trace_cores=trace_cores
    )
    out = np.empty((B, S, D), np.float32)
    for c in range(NCORES):
        b, r = c // 4, c % 4
        out[b, r * SOUT:(r + 1) * SOUT, :] = res.results[c]["out"]
    out += bo[None, None, :]
    return out, res


def kernel(**inputs):
    out, _ = _run(inputs)
    return out
